# revision 28
# baseline (speedup 1.0000x reference)
"""Trainium2 Bass kernel for the BDH-style weight-tied transformer.

Contract: kernel(**inputs) takes FULL unsharded numpy inputs (idx, wte,
encoder, decoder_x, decoder_y, readout) and returns the FULL (B, T, V)
logits, running the model on 8 NeuronCores via run_bass_kernel_spmd.

Sharding: core c -> (b = c // 4, h = c % 4).  Group {0..3} handles batch 0,
{4..7} batch 1.  Within a group: tensor-parallel over heads with AllGather
+ local-sum for (a) the head-summed attention matrix and (b) the second
half of the y @ encoder projection (the first half goes out early as an
AllReduce that hides under the remaining y-phase compute).  LayerNorm is
scale-invariant, so summing heads (instead of averaging) is exact.
Readout is vocab-split 4 ways per group.

The neuron axis of each head is permuted host-side so RoPE pair partners
(2k, 2k+1) live at the same partition of sibling 128-chunks ("even" chunk
2c / "odd" chunk 2c+1).  The rotation then needs no cross-partition data
movement.  The 1/sqrt(d) attention scale is folded into the cos/sin tables
(d**-0.25 on each factor of the Gram matrix).
"""

import sys

for _p in ("/opt/trn_rl_repo", "/opt/pypackages"):
    if _p not in sys.path:
        sys.path.append(_p)

import ml_dtypes
import numpy as np

import concourse.bass as bass
import concourse.mybir as mybir
import concourse.tile as tile
from concourse import bacc
from concourse.bass_utils import run_bass_kernel_spmd

F32 = mybir.dt.float32
BF16 = mybir.dt.bfloat16
I32 = mybir.dt.int32
AX = mybir.AxisListType
ALU = mybir.AluOpType
ACT = mybir.ActivationFunctionType

# Model dims (hardcoded per problem spec)
B, T, D, H, N, V = 2, 256, 256, 4, 32768, 32000
n_head = N // H            # 8192 neurons per head (one core's slice)
P = 128
NCH = n_head // P          # 64 chunks of 128 neurons
NPAIR = NCH // 2           # 32 pair-chunks
L_LAYERS = 6
LN_EPS = 1e-5
ROPE_BASE = 10000.0
VSLICE = V // 4            # 8000 vocab columns per core
VCH = 500                  # vocab chunk (PSUM bank holds 512 f32)
NVCH = VSLICE // VCH       # 16
GX = 8                     # n-chunks per streamed weight group (= rope batch)
NGRP = NCH // GX           # 8 weight groups per phase
S4 = float(n_head) ** -0.25


def _ln_pair(nc, pools, srcs, out):
    """LayerNorm over the free dim (D=256) of two [128, 256] f32 tiles.

    srcs: list of 2 APs (SBUF or PSUM, f32).  out: [128, 2, 256] tile.
    """
    psmall = pools["small"]
    for i, src in enumerate(srcs):
        stats = psmall.tile([P, 6], F32, name=f"ln_st{i}", tag="lnstat")
        nc.vector.bn_stats(stats, src)
        aggr = psmall.tile([P, 2], F32, name=f"ln_ag{i}", tag="lnstat")
        nc.vector.bn_aggr(aggr, stats)
        std = psmall.tile([P, 1], F32, name=f"ln_std{i}", tag="lnstat")
        nc.scalar.activation(std, aggr[:, 1:2], ACT.Sqrt,
                             bias=pools["eps"][:, :1])
        rinv = psmall.tile([P, 1], F32, name=f"ln_rinv{i}", tag="lnstat")
        nc.vector.reciprocal(rinv, std)
        nc.vector.tensor_scalar(out[:, i, :], src, aggr[:, 0:1], rinv,
                                op0=ALU.subtract, op1=ALU.mult)


def _transpose4(nc, pools, src, dst, ident):
    """dst[:, k, 128*i:128*(i+1)] = src[:, i, 128*k:128*(k+1)].T  (bf16).

    src, dst: [128, 2, 256] bf16.  Four PE transposes + ACT copies.
    """
    pwork = pools["ps_work"]
    for i in range(2):
        for k in range(2):
            tp = pwork.tile([P, P], BF16, name=f"tp_{i}_{k}", tag="work")
            nc.tensor.transpose(tp, src[:, i, P * k:P * (k + 1)], ident)
            nc.scalar.copy(dst[:, k, P * i:P * (i + 1)], tp)


def build_nc(num_cores=8):
    nc = bacc.Bacc(
        "TRN2", target_bir_lowering=False, debug=False, num_devices=num_cores
    )

    # ---- DRAM I/O (per-core data supplied via in_maps) ----
    wte_d = nc.dram_tensor("wte", [V, D], F32, kind="ExternalInput").ap()
    idx_d = nc.dram_tensor("idx2", [2, P], I32, kind="ExternalInput").ap()
    wx_d = nc.dram_tensor("wx", [D, n_head], BF16, kind="ExternalInput").ap()
    wy_d = nc.dram_tensor("wy", [D, n_head], BF16, kind="ExternalInput").ap()
    enc_d = nc.dram_tensor("enc", [n_head, D], BF16, kind="ExternalInput").ap()
    ro_d = nc.dram_tensor("ro", [D, VSLICE], BF16, kind="ExternalInput").ap()
    cs_d = nc.dram_tensor("cs", [P, NPAIR, 2 * T], BF16, kind="ExternalInput").ap()
    masks_d = nc.dram_tensor("masks", [P, 2, T], BF16, kind="ExternalInput").ap()
    ident_d = nc.dram_tensor("ident", [P, P], BF16, kind="ExternalInput").ap()
    out_d = nc.dram_tensor("out", [T, VSLICE], F32, kind="ExternalOutput").ap()

    groups = [[0, 1, 2, 3], [4, 5, 6, 7]]

    from contextlib import ExitStack
    with tile.TileContext(nc) as tc:
        with ExitStack() as _stk:
            _e = _stk.enter_context
            pers = _e(tc.tile_pool(name="pers", bufs=1))
            pv = _e(tc.tile_pool(name="pv", bufs=2))
            pbig = _e(tc.tile_pool(name="pbig", bufs=2))
            pwx = _e(tc.tile_pool(name="pwx", bufs=3))
            pwy = _e(tc.tile_pool(name="pwy", bufs=3))
            pro = _e(tc.tile_pool(name="pro", bufs=7))
            pxr = _e(tc.tile_pool(name="pxr", bufs=2))
            py = _e(tc.tile_pool(name="py", bufs=4))
            psmall = _e(tc.tile_pool(name="psmall", bufs=16))
            pexp = _e(tc.tile_pool(name="pexp", bufs=2))
            pg = _e(tc.tile_pool(name="pg", bufs=6))
            ps_work = _e(tc.tile_pool(name="ps_work", bufs=5, space="PSUM"))
            ps_accum = _e(tc.tile_pool(name="ps_accum", bufs=1, space="PSUM"))
            dram = _e(tc.tile_pool(name="dram", bufs=2, space="DRAM"))
            pools = {
                "small": psmall,
                "ps_work": ps_work,
            }

            # ---- persistent SBUF tensors ----
            eps_sb = pers.tile([P, 1], F32, name="eps_sb", tag="eps")
            nc.vector.memset(eps_sb, LN_EPS)
            pools["eps"] = eps_sb
            cs_sb = pers.tile([P, NPAIR, 2 * T], BF16, name="cs_sb", tag="cs")
            masks_sb = pers.tile([P, 2, T], BF16, name="masks_sb", tag="masks")
            ident_sb = pers.tile([P, P], BF16, name="ident_sb", tag="ident")
            enc_sb = pers.tile([P, NCH, T], BF16, name="enc_sb", tag="enc")
            x_sb = pers.tile([P, NCH, T], BF16, name="x_sb", tag="x")

            # ---- embedding gather first: it heads the layer-0 critical path
            # ---- warm-up collectives FIRST on the gpsimd queue: two tiny
            # calls (AG + AR) triggered at t~0 absorb the ~45us ncfw global
            # init under the layer-0 x phase.  Per-instance descriptor staging
            # happens at nrt_load, so the real collectives then run at
            # steady-state cost.
            warm_sb = pexp.tile([P, 16], BF16, name="warm_sb", tag="warm", bufs=1)
            nc.vector.memset(warm_sb, 0.0)
            wa_in = dram.tile([P, 16], BF16, name="wa_in", tag="warm_in")
            nc.gpsimd.dma_start(wa_in[:], warm_sb)
            wa_out = dram.tile([4, P, 16], BF16, name="wa_out", tag="warm_ag")
            nc.gpsimd.collective_compute(
                "AllGather", ALU.bypass, replica_groups=groups,
                ins=[wa_in.opt()], outs=[wa_out.opt()],
            )
            wra_out = dram.tile([P, 16], BF16, name="wra_out", tag="warm_ar")
            nc.gpsimd.collective_compute(
                "AllReduce", ALU.add, replica_groups=groups,
                ins=[wa_in.opt()], outs=[wra_out.opt()],
            )

            vraw = pbig.tile([P, 2, T], F32, name="vraw", tag="vraw", bufs=1)
            for i in range(2):
                idx_sb = psmall.tile([P, 1], I32, name=f"idx_sb{i}", tag="idx")
                nc.sync.dma_start(idx_sb, idx_d[i, :].rearrange("(p o) -> p o", o=1))
                nc.gpsimd.indirect_dma_start(
                    out=vraw[:, i, :],
                    out_offset=None,
                    in_=wte_d[:],
                    in_offset=bass.IndirectOffsetOnAxis(ap=idx_sb[:, :1], axis=0),
                )
            nc.sync.dma_start(masks_sb[:], masks_d[:])
            nc.sync.dma_start(ident_sb[:], ident_d[:])

            def load_cs_group(g):
                # cos/sin for rope batch g (4 pair-chunks), split 4 ways
                for c in range(4):
                    nc.sync.dma_start(
                        cs_sb[:, 4 * g + c:4 * g + c + 1, :],
                        cs_d[:, 4 * g + c:4 * g + c + 1, :],
                    )

            def load_enc_group(g):
                # encoder chunks 8g..8g+7, split 2 ways
                for c in range(2):
                    nc.sync.dma_start(
                        enc_sb[:, 8 * g + 4 * c:8 * g + 4 * (c + 1), :],
                        enc_r[:, 8 * g + 4 * c:8 * g + 4 * (c + 1), :],
                    )

            enc_r = enc_d.rearrange("(c p) d -> p c d", p=P)
            load_cs_group(0)
            load_cs_group(1)

            _kw_id = [0]

            def pe_keepwarm(n=40):
                # Dependency-free matmuls that run during a collective stall so
                # the HAM clock gate keeps the PE at 2.4 GHz (a >3.4us idle
                # re-throttles it to 1.2 GHz for the next ~3.4us of work).
                # Writes ride the yeA accumulator's PSUM bank, which is idle at
                # every keepwarm site (between its flushA read and next-layer
                # yenc writes).
                _kw_id[0] += 1
                jp = ps_accum.tile([P, T], F32, name=f"junk_{_kw_id[0]}",
                                   tag="accC")
                for _ in range(n):
                    nc.tensor.matmul(jp[:, 0:T], lhsT=ident_sb,
                                     rhs=masks_sb[:, 1, :],
                                     start=True, stop=True)

            pe_keepwarm(45)
            v = pv.tile([P, 2, T], F32, name="v_l0", tag="v")
            _ln_pair(nc, pools, [vraw[:, 0, :], vraw[:, 1, :]], v)

            for layer in range(L_LAYERS):
                # ---- v_bf (natural, bf16) and vT (transposed, bf16) ----
                v_bf = pbig.tile([P, 2, T], BF16, name=f"vbf_{layer}", tag="vbf")
                for i in range(2):
                    nc.scalar.copy(v_bf[:, i, :], v[:, i, :])
                vT = pbig.tile([P, 2, T], BF16, name=f"vT_{layer}", tag="vT")
                _transpose4(nc, pools, v_bf, vT, ident_sb)

                # ---- x phase: x = relu(v @ Wx), rope, scores (Gram) ----
                # sc0/sc1 accumulation groups interleave, so they must live in
                # different PSUM banks (start=True owns a whole 2KB zero region)
                sc0 = ps_accum.tile([P, P], F32, name=f"sc0_{layer}", tag="accA")
                sc1 = ps_accum.tile([P, T], F32, name=f"sc1_{layer}", tag="accB")
                scores = [sc0, sc1]

                def emit_scores(grp, xr_e, xr_o):
                    ch0 = GX * grp
                    for q in range(4):  # pair within rope batch
                        for xr in (xr_e, xr_o):
                            chv = ch0 + 2 * q + (0 if xr is xr_e else 1)
                            nc.tensor.matmul(
                                scores[0],
                                lhsT=xr[:, q, 0:P],
                                rhs=xr[:, q, 0:P],
                                start=(chv == 0),
                                stop=(chv == NCH - 1),
                            )
                            nc.tensor.matmul(
                                scores[1],
                                lhsT=xr[:, q, P:2 * P],
                                rhs=xr[:, q, :],
                                start=(chv == 0),
                                stop=(chv == NCH - 1),
                            )

                pending = None  # (grp, xr_e, xr_o) awaiting scores emission
                for grp in range(NGRP):  # 8 groups of 8 chunks (4 rope pairs)
                    ch0 = GX * grp
                    wxg = pwx.tile([P, 2, GX * P], BF16,
                                   name=f"wxg_{layer}_{ch0}", tag="wx")
                    for dk in range(2):
                        for ch in range(2):
                            nc.sync.dma_start(
                                wxg[:, dk, 512 * ch:512 * (ch + 1)],
                                wx_d[P * dk:P * (dk + 1),
                                     P * ch0 + 512 * ch:P * ch0 + 512 * (ch + 1)],
                            )
                    if layer == 0:
                        if grp + 2 < NGRP:
                            load_cs_group(grp + 2)
                    for pc in range(4 * grp, 4 * grp + 4):
                        x_pre = ps_work.tile([P, 2 * T], F32,
                                             name=f"xpre_{layer}_{pc}", tag="work")
                        for m in range(2):  # even / odd member chunk
                            ch = 2 * pc + m
                            co = P * (ch % GX)
                            for dk in range(2):
                                nc.tensor.matmul(
                                    x_pre[:, T * m:T * (m + 1)],
                                    lhsT=wxg[:, dk, co:co + P],
                                    rhs=vT[:, dk, :],
                                    start=(dk == 0),
                                    stop=(dk == 1),
                                )
                        nc.scalar.activation(
                            x_sb[:, 2 * pc:2 * pc + 2, :], x_pre, ACT.Relu)
                    # rope over the 4 pair-chunks of this group, batched FD=1024
                    xe = x_sb[:, ch0:ch0 + GX:2, :]
                    xo = x_sb[:, ch0 + 1:ch0 + GX:2, :]
                    cvw = cs_sb[:, 4 * grp:4 * grp + 4, 0:T]
                    svw = cs_sb[:, 4 * grp:4 * grp + 4, T:2 * T]
                    m_ec = pxr.tile([P, 4, T], BF16, name=f"mec_{layer}_{grp}", tag="m_ec", bufs=1)
                    m_os = pxr.tile([P, 4, T], BF16, name=f"mos_{layer}_{grp}", tag="m_os", bufs=1)
                    m_oc = pxr.tile([P, 4, T], BF16, name=f"moc_{layer}_{grp}", tag="m_oc", bufs=1)
                    m_es = pxr.tile([P, 4, T], BF16, name=f"mes_{layer}_{grp}", tag="m_es", bufs=1)
                    xr_e = pxr.tile([P, 4, T], BF16, name=f"xre_{layer}_{grp}", tag="xr_e")
                    xr_o = pxr.tile([P, 4, T], BF16, name=f"xro_{layer}_{grp}", tag="xr_o")
                    nc.vector.tensor_mul(m_ec, xe, cvw)
                    nc.vector.tensor_mul(m_os, xo, svw)
                    nc.vector.tensor_sub(xr_e, m_ec, m_os)
                    nc.vector.tensor_mul(m_oc, xo, cvw)
                    nc.vector.tensor_mul(m_es, xe, svw)
                    nc.vector.tensor_add(xr_o, m_oc, m_es)
                    if pending is not None:
                        emit_scores(*pending)
                    pending = (grp, xr_e, xr_o)
                emit_scores(*pending)
                pe_keepwarm(10)  # PE filler while softmax runs

                # ---- softmax (causal, per-head normalized) ----
                # attn packed [128, 384]: cols 0:128 = t-tile0 (s<128),
                # cols 128:384 = t-tile1 (s<256)
                attn = pexp.tile([P, 3 * P], BF16, name=f"attn_{layer}", tag="attn")
                for i, (w, lo) in enumerate(((P, 0), (T, P))):
                    # scores are pre-scaled by 1/sqrt(d) via the rope tables and
                    # bounded (~|s|<15 for this weight scale), so exp needs no
                    # max-subtraction; softmax is shift-invariant.  The causal
                    # mask is additive (-30000 on masked entries, exp -> 0) and
                    # the row-sum rides the exp via accum_out.
                    nc.vector.tensor_add(scores[i], scores[i], masks_sb[:, i, 0:w])
                    ex = pexp.tile([P, w], BF16, name=f"ex_{layer}_{i}", tag="ex")
                    rs = psmall.tile([P, 1], F32, name=f"rs_{i}", tag="lnstat")
                    nc.scalar.activation(ex, scores[i], ACT.Exp, accum_out=rs)
                    rcp = psmall.tile([P, 1], F32, name=f"rcp_{i}", tag="lnstat")
                    nc.vector.reciprocal(rcp, rs)
                    nc.vector.tensor_scalar_mul(attn[:, lo:lo + w], ex, rcp)

                # ---- transpose local attn (pre-collective), AllGather over
                # the 4-core group, then sum heads locally ----
                attnT = pexp.tile([P, 3 * P], BF16, name=f"attnT_{layer}", tag="attnT")
                for bi, (alo, tlo) in enumerate(((0, 0), (P, P), (2 * P, 2 * P))):
                    tp = ps_work.tile([P, P], BF16, name=f"tpa_{bi}", tag="work")
                    nc.tensor.transpose(tp, attn[:, alo:alo + P], ident_sb)
                    nc.scalar.copy(attnT[:, tlo:tlo + P], tp)
                attn_bnc = dram.tile([P, 3 * P], BF16,
                                     name=f"attn_bnc_{layer}", tag="attn_in")
                nc.gpsimd.dma_start(attn_bnc[:, 0:P], attnT[:, 0:P])
                nc.gpsimd.dma_start(attn_bnc[:, P:3 * P], attnT[:, P:3 * P])
                attn_gth = dram.tile([4, P, 3 * P], BF16, name=f"attn_gth_{layer}",
                                     tag="attn_out")
                nc.gpsimd.collective_compute(
                    "AllGather", ALU.bypass, replica_groups=groups,
                    ins=[attn_bnc.opt()], outs=[attn_gth.opt()],
                )
                pe_keepwarm(32)
                ga = []
                for r in range(4):
                    g_r = pg.tile([P, 3 * P], BF16, name=f"ga_{layer}_{r}", tag="ag")
                    nc.sync.dma_start(g_r, attn_gth[r])
                    ga.append(g_r)
                s01 = pg.tile([P, 3 * P], BF16, name=f"s01_{layer}", tag="ag")
                nc.vector.tensor_add(s01, ga[0], ga[1])
                s23 = pg.tile([P, 3 * P], BF16, name=f"s23_{layer}", tag="ag")
                nc.vector.tensor_add(s23, ga[2], ga[3])
                asumT = pexp.tile([P, 3 * P], BF16, name=f"asumT_{layer}", tag="asum")
                nc.vector.tensor_add(asumT, s01, s23)

                # ---- a = asumT.T @ v; LN(a) ----
                a_ps = []
                ap_0 = ps_work.tile([P, T], F32, name=f"aps_{layer}_0", tag="work")
                nc.tensor.matmul(ap_0, lhsT=asumT[:, 0:P], rhs=v_bf[:, 0, :],
                                 start=True, stop=True)
                a_ps.append(ap_0)
                ap_1 = ps_work.tile([P, T], F32, name=f"aps_{layer}_1", tag="work")
                for j in range(2):
                    nc.tensor.matmul(
                        ap_1,
                        lhsT=asumT[:, P * (1 + j):P * (2 + j)],
                        rhs=v_bf[:, j, :],
                        start=(j == 0),
                        stop=(j == 1),
                    )
                a_ps.append(ap_1)
                lnA = pbig.tile([P, 2, T], BF16, name=f"lnA_{layer}", tag="lnA")
                _ln_pair(nc, pools, a_ps, lnA)
                lnAT = pbig.tile([P, 2, T], BF16, name=f"lnAT_{layer}", tag="lnAT")
                _transpose4(nc, pools, lnA, lnAT, ident_sb)

                # ---- y phase: y = relu(lnA @ Wy) * x (fused on gpsimd);
                # yenc = y @ enc, reduced in two halves: first half AllReduce
                # (hidden under 2nd-half compute), second half AllGather ----
                # interleaved groups -> distinct banks; yB1 reuses bank A via
                # the tag ring (waits for flushA's read of yA0, which is
                # emitted before the second half's matmuls)
                yA0 = ps_accum.tile([P, T], F32, name=f"yA0_{layer}", tag="accA")
                yA1 = ps_accum.tile([P, T], F32, name=f"yA1_{layer}", tag="accB")
                yB0 = ps_accum.tile([P, T], F32, name=f"yB0_{layer}", tag="accC")
                yB1 = ps_accum.tile([P, T], F32, name=f"yB1_{layer}", tag="accA")
                yhalf = [(yA0, yA1), (yB0, yB1)]

                def emit_yenc(pc, yt):
                    ch0y = 2 * pc
                    half = 0 if ch0y < NCH // 2 else 1
                    base = half * (NCH // 2)
                    acc = yhalf[half]
                    for m in range(2):
                        ch = ch0y + m
                        for i in range(2):
                            nc.tensor.matmul(
                                acc[i],
                                lhsT=yt[:, T * m + P * i:T * m + P * (i + 1)],
                                rhs=enc_sb[:, ch, :],
                                start=(ch == base),
                                stop=(ch == base + NCH // 2 - 1),
                            )

                yeA_gth = None
                pend_y = None
                for pc in range(NCH // 2):  # two n-chunks at a time
                    ch0y = 2 * pc
                    if ch0y % GX == 0:
                        gy = ch0y // GX
                        wyg = pwy.tile([P, 2, GX * P], BF16,
                                       name=f"wyg_{layer}_{ch0y}", tag="wy")
                        for dk in range(2):
                            for ch in range(2):
                                nc.sync.dma_start(
                                    wyg[:, dk, 512 * ch:512 * (ch + 1)],
                                    wy_d[P * dk:P * (dk + 1),
                                         P * ch0y + 512 * ch:P * ch0y + 512 * (ch + 1)],
                                )
                        if layer == 0 and gy == 0:
                            load_enc_group(0)
                        if layer == 0 and gy + 1 < NGRP:
                            load_enc_group(gy + 1)
                    y_pre = ps_work.tile([P, 2 * T], F32, name=f"ypre_{layer}_{pc}",
                                         tag="work")
                    for m in range(2):
                        co = P * ((ch0y + m) % GX)
                        for dk in range(2):
                            nc.tensor.matmul(
                                y_pre[:, T * m:T * (m + 1)],
                                lhsT=wyg[:, dk, co:co + P],
                                rhs=lnAT[:, dk, :],
                                start=(dk == 0),
                                stop=(dk == 1),
                            )
                    # fused y = relu(y_pre) * x in one DVE op (gpsimd cannot
                    # read PSUM); replaces a scalar relu + separate multiply
                    yt = py.tile([P, 2 * T], BF16, name=f"yt_{layer}_{pc}", tag="y")
                    nc.vector.scalar_tensor_tensor(
                        yt, y_pre, 0.0, x_sb[:, ch0y:ch0y + 2, :].rearrange("p c t -> p (c t)"),
                        op0=ALU.max, op1=ALU.mult)
                    if pend_y is not None:
                        emit_yenc(*pend_y)
                        if pend_y[0] == NCH // 4 - 1:
                            # first half done: ship it as AllReduce, hidden
                            # under the second half's compute
                            yeA_sb = pexp.tile([P, 2 * T], BF16,
                                               name=f"yeA_sb_{layer}", tag="yeA")
                            for i in range(2):
                                nc.scalar.copy(yeA_sb[:, T * i:T * (i + 1)], yhalf[0][i])
                            # bounce on the gpsimd queue: the sync queues are
                            # busy streaming wy and would delay the collective
                            yeA_bnc = dram.tile([P, 2 * T], BF16,
                                                name=f"yeA_bnc_{layer}", tag="yeA_in")
                            for i in range(2):
                                nc.gpsimd.dma_start(yeA_bnc[:, T * i:T * (i + 1)],
                                                    yeA_sb[:, T * i:T * (i + 1)])
                            yeA_gth = dram.tile([P, 2 * T], BF16,
                                                name=f"yeA_gth_{layer}", tag="yeA_out")
                            nc.gpsimd.collective_compute(
                                "AllReduce", ALU.add, replica_groups=groups,
                                ins=[yeA_bnc.opt()], outs=[yeA_gth.opt()],
                            )
                    pend_y = (pc, yt)
                emit_yenc(*pend_y)

                # ---- second half: AllGather + local sum; combine with A ----
                yeB_sb = pexp.tile([P, 2 * T], BF16, name=f"yeB_sb_{layer}", tag="yeB")
                for i in range(2):
                    nc.scalar.copy(yeB_sb[:, T * i:T * (i + 1)], yhalf[1][i])
                yeB_bnc = dram.tile([P, 2 * T], BF16, name=f"yeB_bnc_{layer}",
                                    tag="yeB_in")
                for i in range(2):
                    nc.gpsimd.dma_start(yeB_bnc[:, T * i:T * (i + 1)],
                                        yeB_sb[:, T * i:T * (i + 1)])
                yeB_gth = dram.tile([4, P, 2 * T], BF16, name=f"yeB_gth_{layer}",
                                    tag="yeB_out")
                nc.gpsimd.collective_compute(
                    "AllGather", ALU.bypass, replica_groups=groups,
                    ins=[yeB_bnc.opt()], outs=[yeB_gth.opt()],
                )
                pe_keepwarm(32)
                yeA_back = pexp.tile([P, 2 * T], BF16, name=f"yeA_back_{layer}",
                                     tag="yeAb")
                nc.gpsimd.dma_start(yeA_back, yeA_gth[:])
                gb = []
                for r in range(4):
                    gb_r = pg.tile([P, 2 * T], BF16, name=f"gb_{layer}_{r}", tag="bg")
                    nc.sync.dma_start(gb_r, yeB_gth[r])
                    gb.append(gb_r)
                b01 = pg.tile([P, 2 * T], BF16, name=f"b01_{layer}", tag="bg")
                nc.vector.tensor_add(b01, gb[0], gb[1])
                b23 = pg.tile([P, 2 * T], BF16, name=f"b23_{layer}", tag="bg")
                nc.vector.tensor_add(b23, gb[2], gb[3])
                bsum = pg.tile([P, 2 * T], BF16, name=f"bsum_{layer}", tag="bg")
                nc.vector.tensor_add(bsum, b01, b23)
                ysum = pbig.tile([P, 2, T], BF16, name=f"ysum_{layer}", tag="ysum")
                nc.vector.tensor_add(
                    ysum.rearrange("p a t -> p (a t)"), bsum, yeA_back)
                lnY = pbig.tile([P, 2, T], F32, name=f"lnY_{layer}", tag="lnY", bufs=1)
                _ln_pair(nc, pools, [ysum[:, 0, :], ysum[:, 1, :]], lnY)
                vres = pbig.tile([P, 2, T], F32, name=f"vres_{layer}", tag="vres", bufs=1)
                for i in range(2):
                    nc.vector.tensor_add(vres[:, i, :], v[:, i, :], lnY[:, i, :])
                v = pv.tile([P, 2, T], F32, name=f"v_l{layer + 1}", tag="v")
                _ln_pair(nc, pools, [vres[:, 0, :], vres[:, 1, :]], v)

            # ---- readout: out = v @ ro  (vocab slice) ----
            v_bf = pbig.tile([P, 2, T], BF16, name="vbf_ro", tag="vbf")
            for i in range(2):
                nc.scalar.copy(v_bf[:, i, :], v[:, i, :])
            vT = pbig.tile([P, 2, T], BF16, name="vT_ro", tag="vT")
            _transpose4(nc, pools, v_bf, vT, ident_sb)
            for c in range(NVCH):
                rog = pro.tile([P, 2, VCH], BF16, name=f"rog_{c}", tag="ro")
                for dk in range(2):
                    nc.sync.dma_start(
                        rog[:, dk, :],
                        ro_d[P * dk:P * (dk + 1), VCH * c:VCH * (c + 1)],
                    )
                for i in range(2):
                    lg = ps_work.tile([P, VCH], F32, name=f"lg_{c}_{i}", tag="work")
                    for dk in range(2):
                        nc.tensor.matmul(
                            lg,
                            lhsT=vT[:, dk, P * i:P * (i + 1)],
                            rhs=rog[:, dk, :],
                            start=(dk == 0),
                            stop=(dk == 1),
                        )
                    lg_sb = py.tile([P, VCH], F32, name=f"lg_sb_{c}_{i}",
                                    tag="lgsb", bufs=4)
                    nc.vector.tensor_copy(lg_sb, lg)
                    nc.sync.dma_start(
                        out_d[P * i:P * (i + 1), VCH * c:VCH * (c + 1)], lg_sb
                    )

    nc.compile()
    return nc


# ------------------------- host-side preparation -------------------------

def _pair_perm():
    """perm[new] = old index within a head, de-interleaving rope pairs."""
    perm = np.zeros(n_head, dtype=np.int64)
    for c in range(NPAIR):
        k = np.arange(P) + c * P          # pair indices in this pair-chunk
        perm[(2 * c) * P + np.arange(P)] = 2 * k
        perm[(2 * c + 1) * P + np.arange(P)] = 2 * k + 1
    return perm


def _rope_tables():
    """cs[p, c, 0:T] = cos, cs[p, c, T:2T] = sin, scaled by d**-0.25."""
    inv_freq = 1.0 / (
        ROPE_BASE ** (np.arange(0, n_head, 2, dtype=np.float32) / n_head)
    )  # (4096,) f32, matching reference arithmetic
    t = np.arange(T, dtype=np.float32)
    freqs = t[:, None] * inv_freq[None, :]         # (T, 4096) f32
    cos = np.cos(freqs) * S4                       # (T, 4096)
    sin = np.sin(freqs) * S4
    cs = np.zeros((P, NPAIR, 2 * T), dtype=np.float32)
    for c in range(NPAIR):
        k = c * P + np.arange(P)                   # (128,) pair indices
        cs[:, c, 0:T] = cos[:, k].T
        cs[:, c, T:2 * T] = sin[:, k].T
    return cs.astype(ml_dtypes.bfloat16)


def _masks():
    # Additive causal mask [P, 2, T]: 0 on allowed entries, -30000 on masked
    # (exp underflows to 0).  tile0 in [:, 0, 0:128] (s<=t); tile1 [:, 1, :].
    m = np.full((P, 2, T), -30000.0, dtype=np.float32)
    t = np.arange(P)[:, None]
    m[:, 0, 0:P] = np.where(np.arange(P)[None, :] <= t, 0.0, -30000.0)
    m[:, 1, :] = np.where(np.arange(T)[None, :] <= t + P, 0.0, -30000.0)
    return m.astype(ml_dtypes.bfloat16)


_CACHE = {}


def kernel(idx, wte, encoder, decoder_x, decoder_y, readout):
    if "nc" not in _CACHE:
        _CACHE["nc"] = build_nc()
    nc = _CACHE["nc"]
    in_maps = prepare_in_maps(idx, wte, encoder, decoder_x, decoder_y, readout)
    res = run_bass_kernel_spmd(nc, in_maps, core_ids=list(range(8)))
    return assemble_output([res.results[c]["out"] for c in range(8)])


def assemble_output(outs):
    out = np.empty((B, T, V), dtype=np.float32)
    for c in range(8):
        b, h = c // 4, c % 4
        out[b, :, h * VSLICE:(h + 1) * VSLICE] = outs[c]
    return out


def prepare_in_maps(idx, wte, encoder, decoder_x, decoder_y, readout):
    idx = np.asarray(idx)
    wte = np.ascontiguousarray(np.asarray(wte, dtype=np.float32))
    encoder = np.asarray(encoder, dtype=np.float32)
    decoder_x = np.asarray(decoder_x, dtype=np.float32)
    decoder_y = np.asarray(decoder_y, dtype=np.float32)
    readout = np.asarray(readout, dtype=np.float32)

    perm = _pair_perm()
    cs = _rope_tables()
    masks = _masks()
    ident = np.eye(P, dtype=np.float32).astype(ml_dtypes.bfloat16)
    bf = ml_dtypes.bfloat16

    wx_h = [np.ascontiguousarray(decoder_x[h][:, perm].astype(bf)) for h in range(H)]
    wy_h = [np.ascontiguousarray(decoder_y[h][:, perm].astype(bf)) for h in range(H)]
    enc_h = [
        np.ascontiguousarray(encoder[h * n_head + perm, :].astype(bf))
        for h in range(H)
    ]
    ro_h = [
        np.ascontiguousarray(readout[:, h * VSLICE:(h + 1) * VSLICE].astype(bf))
        for h in range(H)
    ]
    idx_b = [np.ascontiguousarray(idx[b].reshape(2, P).astype(np.int32))
             for b in range(B)]

    in_maps = []
    for c in range(8):
        b, h = c // 4, c % 4
        in_maps.append({
            "wte": wte,
            "idx2": idx_b[b],
            "wx": wx_h[h],
            "wy": wy_h[h],
            "enc": enc_h[h],
            "ro": ro_h[h],
            "cs": cs,
            "masks": masks,
            "ident": ident,
        })

    return in_maps


if __name__ == "__main__":
    nc = build_nc()
    print("built + compiled OK")


# revision 34
# speedup vs baseline: 1.0738x; 1.0738x over previous
"""Trainium2 Bass kernel for the BDH-style weight-tied transformer.

Contract: kernel(**inputs) takes FULL unsharded numpy inputs (idx, wte,
encoder, decoder_x, decoder_y, readout) and returns the FULL (B, T, V)
logits, running the model on 8 NeuronCores via run_bass_kernel_spmd.

Sharding: core c -> (b = c // 4, h = c % 4).  Group {0..3} handles batch 0,
{4..7} batch 1.  Within a group: tensor-parallel over heads with AllGather
+ local-sum for (a) the head-summed attention matrix and (b) the second
half of the y @ encoder projection (the first half goes out early as an
AllReduce that hides under the remaining y-phase compute).  LayerNorm is
scale-invariant, so summing heads (instead of averaging) is exact.
Readout is vocab-split 4 ways per group.

The neuron axis of each head is permuted host-side so RoPE pair partners
(2k, 2k+1) live at the same partition of sibling 128-chunks ("even" chunk
2c / "odd" chunk 2c+1).  The rotation then needs no cross-partition data
movement.  The 1/sqrt(d) attention scale is folded into the cos/sin tables
(d**-0.25 on each factor of the Gram matrix).
"""

import sys

for _p in ("/opt/trn_rl_repo", "/opt/pypackages"):
    if _p not in sys.path:
        sys.path.append(_p)

import ml_dtypes
import numpy as np

import concourse.bass as bass
import concourse.mybir as mybir
import concourse.tile as tile
from concourse import bacc
from concourse.bass_utils import run_bass_kernel_spmd

F32 = mybir.dt.float32
BF16 = mybir.dt.bfloat16
I32 = mybir.dt.int32
AX = mybir.AxisListType
ALU = mybir.AluOpType
ACT = mybir.ActivationFunctionType

# Model dims (hardcoded per problem spec)
B, T, D, H, N, V = 2, 256, 256, 4, 32768, 32000
n_head = N // H            # 8192 neurons per head (one core's slice)
P = 128
NCH = n_head // P          # 64 chunks of 128 neurons
NPAIR = NCH // 2           # 32 pair-chunks
L_LAYERS = 6
LN_EPS = 1e-5
ROPE_BASE = 10000.0
VSLICE = V // 4            # 8000 vocab columns per core
VCH = 500                  # vocab chunk (PSUM bank holds 512 f32)
NVCH = VSLICE // VCH       # 16
GX = 8                     # n-chunks per streamed weight group (= rope batch)
NGRP = NCH // GX           # 8 weight groups per phase
S4 = float(n_head) ** -0.25


def _ln_pair(nc, pools, srcs, out):
    """LayerNorm over the free dim (D=256) of two [128, 256] f32 tiles.

    srcs: list of 2 APs (SBUF or PSUM, f32).  out: [128, 2, 256] tile.
    """
    psmall = pools["small"]
    for i, src in enumerate(srcs):
        stats = psmall.tile([P, 6], F32, name=f"ln_st{i}", tag="lnstat")
        nc.vector.bn_stats(stats, src)
        aggr = psmall.tile([P, 2], F32, name=f"ln_ag{i}", tag="lnstat")
        nc.vector.bn_aggr(aggr, stats)
        std = psmall.tile([P, 1], F32, name=f"ln_std{i}", tag="lnstat")
        nc.scalar.activation(std, aggr[:, 1:2], ACT.Sqrt,
                             bias=pools["eps"][:, :1])
        rinv = psmall.tile([P, 1], F32, name=f"ln_rinv{i}", tag="lnstat")
        nc.vector.reciprocal(rinv, std)
        nc.vector.tensor_scalar(out[:, i, :], src, aggr[:, 0:1], rinv,
                                op0=ALU.subtract, op1=ALU.mult)


def _transpose4(nc, pools, src, dst, ident):
    """dst[:, k, 128*i:128*(i+1)] = src[:, i, 128*k:128*(k+1)].T  (bf16).

    src, dst: [128, 2, 256] bf16.  Four PE transposes + ACT copies.
    """
    pwork = pools["ps_work"]
    for i in range(2):
        for k in range(2):
            tp = pwork.tile([P, P], BF16, name=f"tp_{i}_{k}", tag="work")
            nc.tensor.transpose(tp, src[:, i, P * k:P * (k + 1)], ident)
            nc.scalar.copy(dst[:, k, P * i:P * (i + 1)], tp)


def build_nc(num_cores=8):
    nc = bacc.Bacc(
        "TRN2", target_bir_lowering=False, debug=False, num_devices=num_cores
    )

    # ---- DRAM I/O (per-core data supplied via in_maps) ----
    wte_d = nc.dram_tensor("wte", [V, D], F32, kind="ExternalInput").ap()
    idx_d = nc.dram_tensor("idx2", [2, P], I32, kind="ExternalInput").ap()
    wx_d = nc.dram_tensor("wx", [D, n_head], BF16, kind="ExternalInput").ap()
    wy_d = nc.dram_tensor("wy", [D, n_head], BF16, kind="ExternalInput").ap()
    enc_d = nc.dram_tensor("enc", [n_head, D], BF16, kind="ExternalInput").ap()
    ro_d = nc.dram_tensor("ro", [D, VSLICE], BF16, kind="ExternalInput").ap()
    cs_d = nc.dram_tensor("cs", [P, NPAIR, 2 * T], BF16, kind="ExternalInput").ap()
    masks_d = nc.dram_tensor("masks", [P, 2, T], BF16, kind="ExternalInput").ap()
    ident_d = nc.dram_tensor("ident", [P, P], BF16, kind="ExternalInput").ap()
    out_d = nc.dram_tensor("out", [T, VSLICE], F32, kind="ExternalOutput").ap()

    groups = [[0, 1, 2, 3], [4, 5, 6, 7]]

    from contextlib import ExitStack
    with tile.TileContext(nc) as tc:
        with ExitStack() as _stk:
            _e = _stk.enter_context
            pers = _e(tc.tile_pool(name="pers", bufs=1))
            pv = _e(tc.tile_pool(name="pv", bufs=2))
            pbig = _e(tc.tile_pool(name="pbig", bufs=2))
            pwx = _e(tc.tile_pool(name="pwx", bufs=3))
            pwy = _e(tc.tile_pool(name="pwy", bufs=3))
            pro = _e(tc.tile_pool(name="pro", bufs=7))
            pxr = _e(tc.tile_pool(name="pxr", bufs=2))
            py = _e(tc.tile_pool(name="py", bufs=4))
            psmall = _e(tc.tile_pool(name="psmall", bufs=16))
            pexp = _e(tc.tile_pool(name="pexp", bufs=2))
            pg = _e(tc.tile_pool(name="pg", bufs=6))
            ps_work = _e(tc.tile_pool(name="ps_work", bufs=5, space="PSUM"))
            ps_accum = _e(tc.tile_pool(name="ps_accum", bufs=1, space="PSUM"))
            dram = _e(tc.tile_pool(name="dram", bufs=2, space="DRAM"))
            pools = {
                "small": psmall,
                "ps_work": ps_work,
            }

            # ---- persistent SBUF tensors ----
            eps_sb = pers.tile([P, 1], F32, name="eps_sb", tag="eps")
            nc.vector.memset(eps_sb, LN_EPS)
            pools["eps"] = eps_sb
            cs_sb = pers.tile([P, NPAIR, 2 * T], BF16, name="cs_sb", tag="cs")
            masks_sb = pers.tile([P, 2, T], BF16, name="masks_sb", tag="masks")
            ident_sb = pers.tile([P, P], BF16, name="ident_sb", tag="ident")
            enc_sb = pers.tile([P, NCH, T], BF16, name="enc_sb", tag="enc")
            x_sb = pers.tile([P, NCH, T], BF16, name="x_sb", tag="x")

            # ---- embedding gather first: it heads the layer-0 critical path
            # (no warm-up collectives: the ncfw global init (~65us wall) hides
            # under the layer-0 x phase before the first real AllGather)
            vraw = pbig.tile([P, 2, T], F32, name="vraw", tag="vraw", bufs=1)
            for i in range(2):
                idx_sb = psmall.tile([P, 1], I32, name=f"idx_sb{i}", tag="idx")
                nc.sync.dma_start(idx_sb, idx_d[i, :].rearrange("(p o) -> p o", o=1))
                nc.gpsimd.indirect_dma_start(
                    out=vraw[:, i, :],
                    out_offset=None,
                    in_=wte_d[:],
                    in_offset=bass.IndirectOffsetOnAxis(ap=idx_sb[:, :1], axis=0),
                )
            nc.sync.dma_start(masks_sb[:], masks_d[:])
            nc.sync.dma_start(ident_sb[:], ident_d[:])

            def load_cs_group(g):
                # cos/sin for rope batch g (4 pair-chunks), split 4 ways
                for c in range(4):
                    nc.sync.dma_start(
                        cs_sb[:, 4 * g + c:4 * g + c + 1, :],
                        cs_d[:, 4 * g + c:4 * g + c + 1, :],
                    )

            def load_enc_group(g):
                # encoder chunks 8g..8g+7, split 2 ways
                for c in range(2):
                    nc.sync.dma_start(
                        enc_sb[:, 8 * g + 4 * c:8 * g + 4 * (c + 1), :],
                        enc_r[:, 8 * g + 4 * c:8 * g + 4 * (c + 1), :],
                    )

            enc_r = enc_d.rearrange("(c p) d -> p c d", p=P)
            load_cs_group(0)
            load_cs_group(1)

            _kw_id = [0]

            def pe_keepwarm(n=40):
                # Dependency-free matmuls that run during a collective stall so
                # the HAM clock gate keeps the PE at 2.4 GHz (a >3.4us idle
                # re-throttles it to 1.2 GHz for the next ~3.4us of work).
                # Writes ride the yeA accumulator's PSUM bank, which is idle at
                # every keepwarm site (between its flushA read and next-layer
                # yenc writes).
                _kw_id[0] += 1
                jp = ps_accum.tile([P, T], F32, name=f"junk_{_kw_id[0]}",
                                   tag="accC")
                for _ in range(n):
                    nc.tensor.matmul(jp[:, 0:T], lhsT=ident_sb,
                                     rhs=masks_sb[:, 1, :],
                                     start=True, stop=True)

            pe_keepwarm(40)
            v = pv.tile([P, 2, T], F32, name="v_l0", tag="v")
            _ln_pair(nc, pools, [vraw[:, 0, :], vraw[:, 1, :]], v)

            for layer in range(L_LAYERS):
                # ---- v_bf (natural, bf16) and vT (transposed, bf16) ----
                v_bf = pbig.tile([P, 2, T], BF16, name=f"vbf_{layer}", tag="vbf")
                for i in range(2):
                    nc.scalar.copy(v_bf[:, i, :], v[:, i, :])
                vT = pbig.tile([P, 2, T], BF16, name=f"vT_{layer}", tag="vT")
                _transpose4(nc, pools, v_bf, vT, ident_sb)

                # ---- x phase: x = relu(v @ Wx), rope, scores (Gram) ----
                # sc0/sc1 accumulation groups interleave, so they must live in
                # different PSUM banks (start=True owns a whole 2KB zero region)
                sc0 = ps_accum.tile([P, P], F32, name=f"sc0_{layer}", tag="accA")
                sc1 = ps_accum.tile([P, T], F32, name=f"sc1_{layer}", tag="accB")
                scores = [sc0, sc1]

                def emit_scores(grp, xr_e, xr_o):
                    ch0 = GX * grp
                    for q in range(4):  # pair within rope batch
                        for xr in (xr_e, xr_o):
                            chv = ch0 + 2 * q + (0 if xr is xr_e else 1)
                            nc.tensor.matmul(
                                scores[0],
                                lhsT=xr[:, q, 0:P],
                                rhs=xr[:, q, 0:P],
                                start=(chv == 0),
                                stop=(chv == NCH - 1),
                            )
                            nc.tensor.matmul(
                                scores[1],
                                lhsT=xr[:, q, P:2 * P],
                                rhs=xr[:, q, :],
                                start=(chv == 0),
                                stop=(chv == NCH - 1),
                            )

                pending = []  # (grp, xr_e, xr_o) awaiting scores emission
                for grp in range(NGRP):  # 8 groups of 8 chunks (4 rope pairs)
                    ch0 = GX * grp
                    wxg = pwx.tile([P, 2, GX * P], BF16,
                                   name=f"wxg_{layer}_{ch0}", tag="wx")
                    for dk in range(2):
                        for ch in range(2):
                            nc.sync.dma_start(
                                wxg[:, dk, 512 * ch:512 * (ch + 1)],
                                wx_d[P * dk:P * (dk + 1),
                                     P * ch0 + 512 * ch:P * ch0 + 512 * (ch + 1)],
                            )
                    if layer == 0:
                        if grp + 2 < NGRP:
                            load_cs_group(grp + 2)
                    for pc in range(4 * grp, 4 * grp + 4):
                        x_pre = ps_work.tile([P, 2 * T], F32,
                                             name=f"xpre_{layer}_{pc}", tag="work")
                        for m in range(2):  # even / odd member chunk
                            ch = 2 * pc + m
                            co = P * (ch % GX)
                            for dk in range(2):
                                nc.tensor.matmul(
                                    x_pre[:, T * m:T * (m + 1)],
                                    lhsT=wxg[:, dk, co:co + P],
                                    rhs=vT[:, dk, :],
                                    start=(dk == 0),
                                    stop=(dk == 1),
                                )
                        nc.scalar.activation(
                            x_sb[:, 2 * pc:2 * pc + 2, :], x_pre, ACT.Relu)
                    # rope over the 4 pair-chunks of this group, batched FD=1024
                    xe = x_sb[:, ch0:ch0 + GX:2, :]
                    xo = x_sb[:, ch0 + 1:ch0 + GX:2, :]
                    cvw = cs_sb[:, 4 * grp:4 * grp + 4, 0:T]
                    svw = cs_sb[:, 4 * grp:4 * grp + 4, T:2 * T]
                    m_ec = pxr.tile([P, 4, T], BF16, name=f"mec_{layer}_{grp}", tag="m_ec", bufs=1)
                    m_os = pxr.tile([P, 4, T], BF16, name=f"mos_{layer}_{grp}", tag="m_os", bufs=1)
                    m_oc = pxr.tile([P, 4, T], BF16, name=f"moc_{layer}_{grp}", tag="m_oc", bufs=1)
                    m_es = pxr.tile([P, 4, T], BF16, name=f"mes_{layer}_{grp}", tag="m_es", bufs=1)
                    xr_e = pxr.tile([P, 4, T], BF16, name=f"xre_{layer}_{grp}", tag="xr_e", bufs=3)
                    xr_o = pxr.tile([P, 4, T], BF16, name=f"xro_{layer}_{grp}", tag="xr_o", bufs=3)
                    nc.vector.tensor_mul(m_ec, xe, cvw)
                    nc.vector.tensor_mul(m_os, xo, svw)
                    nc.vector.tensor_sub(xr_e, m_ec, m_os)
                    nc.vector.tensor_mul(m_oc, xo, cvw)
                    nc.vector.tensor_mul(m_es, xe, svw)
                    nc.vector.tensor_add(xr_o, m_oc, m_es)
                    # 2-group pipeline depth: the relu->rope chain latency
                    # (~5.7us) exceeds the PE group period (~4us), so scores
                    # for group g are emitted while group g+2 computes
                    pending.append((grp, xr_e, xr_o))
                    if len(pending) > 2:
                        emit_scores(*pending.pop(0))
                for p in pending:
                    emit_scores(*p)
                pe_keepwarm(6)  # PE filler while softmax runs

                # ---- softmax (causal, per-head normalized) ----
                # attn packed [128, 384]: cols 0:128 = t-tile0 (s<128),
                # cols 128:384 = t-tile1 (s<256)
                attn = pexp.tile([P, 3 * P], BF16, name=f"attn_{layer}", tag="attn")
                for i, (w, lo) in enumerate(((P, 0), (T, P))):
                    # scores are pre-scaled by 1/sqrt(d) via the rope tables and
                    # bounded (~|s|<15 for this weight scale), so exp needs no
                    # max-subtraction; softmax is shift-invariant.  The causal
                    # mask is additive (-30000 on masked entries, exp -> 0) and
                    # the row-sum rides the exp via accum_out.
                    nc.vector.tensor_add(scores[i], scores[i], masks_sb[:, i, 0:w])
                    ex = pexp.tile([P, w], BF16, name=f"ex_{layer}_{i}", tag="ex")
                    rs = psmall.tile([P, 1], F32, name=f"rs_{i}", tag="lnstat")
                    nc.scalar.activation(ex, scores[i], ACT.Exp, accum_out=rs)
                    rcp = psmall.tile([P, 1], F32, name=f"rcp_{i}", tag="lnstat")
                    nc.vector.reciprocal(rcp, rs)
                    nc.vector.tensor_scalar_mul(attn[:, lo:lo + w], ex, rcp)

                # ---- transpose local attn (pre-collective), AllGather over
                # the 4-core group, then sum heads locally ----
                attnT = pexp.tile([P, 3 * P], BF16, name=f"attnT_{layer}", tag="attnT")
                for bi, (alo, tlo) in enumerate(((0, 0), (P, P), (2 * P, 2 * P))):
                    tp = ps_work.tile([P, P], BF16, name=f"tpa_{bi}", tag="work")
                    nc.tensor.transpose(tp, attn[:, alo:alo + P], ident_sb)
                    nc.scalar.copy(attnT[:, tlo:tlo + P], tp)
                attn_bnc = dram.tile([P, 3 * P], BF16,
                                     name=f"attn_bnc_{layer}", tag="attn_in")
                nc.gpsimd.dma_start(attn_bnc[:, 0:P], attnT[:, 0:P])
                nc.gpsimd.dma_start(attn_bnc[:, P:3 * P], attnT[:, P:3 * P])
                attn_gth = dram.tile([4, P, 3 * P], BF16, name=f"attn_gth_{layer}",
                                     tag="attn_out")
                nc.gpsimd.collective_compute(
                    "AllGather", ALU.bypass, replica_groups=groups,
                    ins=[attn_bnc.opt()], outs=[attn_gth.opt()],
                )
                pe_keepwarm(20)
                ga = []
                for r in range(4):
                    g_r = pg.tile([P, 3 * P], BF16, name=f"ga_{layer}_{r}", tag="ag")
                    nc.sync.dma_start(g_r, attn_gth[r])
                    ga.append(g_r)
                s01 = pg.tile([P, 3 * P], BF16, name=f"s01_{layer}", tag="ag")
                nc.vector.tensor_add(s01, ga[0], ga[1])
                s23 = pg.tile([P, 3 * P], BF16, name=f"s23_{layer}", tag="ag")
                nc.vector.tensor_add(s23, ga[2], ga[3])
                asumT = pexp.tile([P, 3 * P], BF16, name=f"asumT_{layer}", tag="asum")
                nc.vector.tensor_add(asumT, s01, s23)

                # ---- a = asumT.T @ v; LN(a) ----
                a_ps = []
                ap_0 = ps_work.tile([P, T], F32, name=f"aps_{layer}_0", tag="work")
                nc.tensor.matmul(ap_0, lhsT=asumT[:, 0:P], rhs=v_bf[:, 0, :],
                                 start=True, stop=True)
                a_ps.append(ap_0)
                ap_1 = ps_work.tile([P, T], F32, name=f"aps_{layer}_1", tag="work")
                for j in range(2):
                    nc.tensor.matmul(
                        ap_1,
                        lhsT=asumT[:, P * (1 + j):P * (2 + j)],
                        rhs=v_bf[:, j, :],
                        start=(j == 0),
                        stop=(j == 1),
                    )
                a_ps.append(ap_1)
                lnA = pbig.tile([P, 2, T], BF16, name=f"lnA_{layer}", tag="lnA")
                _ln_pair(nc, pools, a_ps, lnA)
                lnAT = pbig.tile([P, 2, T], BF16, name=f"lnAT_{layer}", tag="lnAT")
                _transpose4(nc, pools, lnA, lnAT, ident_sb)

                # ---- y phase: y = relu(lnA @ Wy) * x (fused DVE op);
                # yenc = y @ enc accumulated over all chunks, then one
                # AllReduce over the 4-core group ----
                ye0 = ps_accum.tile([P, T], F32, name=f"ye0_{layer}", tag="accA")
                ye1 = ps_accum.tile([P, T], F32, name=f"ye1_{layer}", tag="accB")
                yacc = (ye0, ye1)

                def emit_yenc(pc, yt):
                    ch0y = 2 * pc
                    for m in range(2):
                        ch = ch0y + m
                        for i in range(2):
                            nc.tensor.matmul(
                                yacc[i],
                                lhsT=yt[:, T * m + P * i:T * m + P * (i + 1)],
                                rhs=enc_sb[:, ch, :],
                                start=(ch == 0),
                                stop=(ch == NCH - 1),
                            )

                pend_y = []
                for pc in range(NCH // 2):  # two n-chunks at a time
                    ch0y = 2 * pc
                    if ch0y % GX == 0:
                        gy = ch0y // GX
                        wyg = pwy.tile([P, 2, GX * P], BF16,
                                       name=f"wyg_{layer}_{ch0y}", tag="wy")
                        for dk in range(2):
                            for ch in range(2):
                                nc.sync.dma_start(
                                    wyg[:, dk, 512 * ch:512 * (ch + 1)],
                                    wy_d[P * dk:P * (dk + 1),
                                         P * ch0y + 512 * ch:P * ch0y + 512 * (ch + 1)],
                                )
                        if layer == 0 and gy == 0:
                            load_enc_group(0)
                        if layer == 0 and gy + 1 < NGRP:
                            load_enc_group(gy + 1)
                    y_pre = ps_work.tile([P, 2 * T], F32, name=f"ypre_{layer}_{pc}",
                                         tag="work")
                    for m in range(2):
                        co = P * ((ch0y + m) % GX)
                        for dk in range(2):
                            nc.tensor.matmul(
                                y_pre[:, T * m:T * (m + 1)],
                                lhsT=wyg[:, dk, co:co + P],
                                rhs=lnAT[:, dk, :],
                                start=(dk == 0),
                                stop=(dk == 1),
                            )
                    # fused y = relu(y_pre) * x in one DVE op; yenc emission
                    # lags 2 chunks so the PE->DVE->PE chain stays pipelined
                    yt = py.tile([P, 2 * T], BF16, name=f"yt_{layer}_{pc}", tag="y")
                    nc.vector.scalar_tensor_tensor(
                        yt, y_pre, 0.0, x_sb[:, ch0y:ch0y + 2, :].rearrange("p c t -> p (c t)"),
                        op0=ALU.max, op1=ALU.mult)
                    pend_y.append((pc, yt))
                    if len(pend_y) > 2:
                        emit_yenc(*pend_y.pop(0))
                for p in pend_y:
                    emit_yenc(*p)

                # ---- AllReduce yenc partials; bounce split across the idle
                # gpsimd + scalar DMA queues (sync queues hold the wy tail) ----
                ye_sb = pexp.tile([P, 2 * T], BF16, name=f"ye_sb_{layer}", tag="yeA")
                for i in range(2):
                    nc.scalar.copy(ye_sb[:, T * i:T * (i + 1)], yacc[i])
                ye_bnc = dram.tile([P, 2 * T], BF16, name=f"ye_bnc_{layer}",
                                   tag="yeA_in")
                nc.gpsimd.dma_start(ye_bnc[:, 0:T], ye_sb[:, 0:T])
                nc.scalar.dma_start(ye_bnc[:, T:2 * T], ye_sb[:, T:2 * T])
                ye_gth = dram.tile([P, 2 * T], BF16, name=f"ye_gth_{layer}",
                                   tag="yeA_out")
                nc.gpsimd.collective_compute(
                    "AllReduce", ALU.add, replica_groups=groups,
                    ins=[ye_bnc.opt()], outs=[ye_gth.opt()],
                )
                pe_keepwarm(20)
                ysum = pbig.tile([P, 2, T], BF16, name=f"ysum_{layer}", tag="ysum")
                nc.sync.dma_start(ysum.rearrange("p a t -> p (a t)"), ye_gth[:])
                lnY = pbig.tile([P, 2, T], F32, name=f"lnY_{layer}", tag="lnY", bufs=1)
                _ln_pair(nc, pools, [ysum[:, 0, :], ysum[:, 1, :]], lnY)
                vres = pbig.tile([P, 2, T], F32, name=f"vres_{layer}", tag="vres", bufs=1)
                for i in range(2):
                    nc.vector.tensor_add(vres[:, i, :], v[:, i, :], lnY[:, i, :])
                v = pv.tile([P, 2, T], F32, name=f"v_l{layer + 1}", tag="v")
                _ln_pair(nc, pools, [vres[:, 0, :], vres[:, 1, :]], v)

            # ---- readout: out = v @ ro  (vocab slice) ----
            v_bf = pbig.tile([P, 2, T], BF16, name="vbf_ro", tag="vbf")
            for i in range(2):
                nc.scalar.copy(v_bf[:, i, :], v[:, i, :])
            vT = pbig.tile([P, 2, T], BF16, name="vT_ro", tag="vT")
            _transpose4(nc, pools, v_bf, vT, ident_sb)
            for c in range(NVCH):
                rog = pro.tile([P, 2, VCH], BF16, name=f"rog_{c}", tag="ro")
                for dk in range(2):
                    nc.sync.dma_start(
                        rog[:, dk, :],
                        ro_d[P * dk:P * (dk + 1), VCH * c:VCH * (c + 1)],
                    )
                for i in range(2):
                    lg = ps_work.tile([P, VCH], F32, name=f"lg_{c}_{i}", tag="work")
                    for dk in range(2):
                        nc.tensor.matmul(
                            lg,
                            lhsT=vT[:, dk, P * i:P * (i + 1)],
                            rhs=rog[:, dk, :],
                            start=(dk == 0),
                            stop=(dk == 1),
                        )
                    lg_sb = py.tile([P, VCH], F32, name=f"lg_sb_{c}_{i}",
                                    tag="lgsb", bufs=4)
                    nc.vector.tensor_copy(lg_sb, lg)
                    nc.sync.dma_start(
                        out_d[P * i:P * (i + 1), VCH * c:VCH * (c + 1)], lg_sb
                    )

    nc.compile()
    return nc


# ------------------------- host-side preparation -------------------------

def _pair_perm():
    """perm[new] = old index within a head, de-interleaving rope pairs."""
    perm = np.zeros(n_head, dtype=np.int64)
    for c in range(NPAIR):
        k = np.arange(P) + c * P          # pair indices in this pair-chunk
        perm[(2 * c) * P + np.arange(P)] = 2 * k
        perm[(2 * c + 1) * P + np.arange(P)] = 2 * k + 1
    return perm


def _rope_tables():
    """cs[p, c, 0:T] = cos, cs[p, c, T:2T] = sin, scaled by d**-0.25."""
    inv_freq = 1.0 / (
        ROPE_BASE ** (np.arange(0, n_head, 2, dtype=np.float32) / n_head)
    )  # (4096,) f32, matching reference arithmetic
    t = np.arange(T, dtype=np.float32)
    freqs = t[:, None] * inv_freq[None, :]         # (T, 4096) f32
    cos = np.cos(freqs) * S4                       # (T, 4096)
    sin = np.sin(freqs) * S4
    cs = np.zeros((P, NPAIR, 2 * T), dtype=np.float32)
    for c in range(NPAIR):
        k = c * P + np.arange(P)                   # (128,) pair indices
        cs[:, c, 0:T] = cos[:, k].T
        cs[:, c, T:2 * T] = sin[:, k].T
    return cs.astype(ml_dtypes.bfloat16)


def _masks():
    # Additive causal mask [P, 2, T]: 0 on allowed entries, -30000 on masked
    # (exp underflows to 0).  tile0 in [:, 0, 0:128] (s<=t); tile1 [:, 1, :].
    m = np.full((P, 2, T), -30000.0, dtype=np.float32)
    t = np.arange(P)[:, None]
    m[:, 0, 0:P] = np.where(np.arange(P)[None, :] <= t, 0.0, -30000.0)
    m[:, 1, :] = np.where(np.arange(T)[None, :] <= t + P, 0.0, -30000.0)
    return m.astype(ml_dtypes.bfloat16)


_CACHE = {}


def kernel(idx, wte, encoder, decoder_x, decoder_y, readout):
    if "nc" not in _CACHE:
        _CACHE["nc"] = build_nc()
    nc = _CACHE["nc"]
    in_maps = prepare_in_maps(idx, wte, encoder, decoder_x, decoder_y, readout)
    res = run_bass_kernel_spmd(nc, in_maps, core_ids=list(range(8)))
    return assemble_output([res.results[c]["out"] for c in range(8)])


def assemble_output(outs):
    out = np.empty((B, T, V), dtype=np.float32)
    for c in range(8):
        b, h = c // 4, c % 4
        out[b, :, h * VSLICE:(h + 1) * VSLICE] = outs[c]
    return out


def prepare_in_maps(idx, wte, encoder, decoder_x, decoder_y, readout):
    idx = np.asarray(idx)
    wte = np.ascontiguousarray(np.asarray(wte, dtype=np.float32))
    encoder = np.asarray(encoder, dtype=np.float32)
    decoder_x = np.asarray(decoder_x, dtype=np.float32)
    decoder_y = np.asarray(decoder_y, dtype=np.float32)
    readout = np.asarray(readout, dtype=np.float32)

    perm = _pair_perm()
    cs = _rope_tables()
    masks = _masks()
    ident = np.eye(P, dtype=np.float32).astype(ml_dtypes.bfloat16)
    bf = ml_dtypes.bfloat16

    wx_h = [np.ascontiguousarray(decoder_x[h][:, perm].astype(bf)) for h in range(H)]
    wy_h = [np.ascontiguousarray(decoder_y[h][:, perm].astype(bf)) for h in range(H)]
    enc_h = [
        np.ascontiguousarray(encoder[h * n_head + perm, :].astype(bf))
        for h in range(H)
    ]
    ro_h = [
        np.ascontiguousarray(readout[:, h * VSLICE:(h + 1) * VSLICE].astype(bf))
        for h in range(H)
    ]
    idx_b = [np.ascontiguousarray(idx[b].reshape(2, P).astype(np.int32))
             for b in range(B)]

    in_maps = []
    for c in range(8):
        b, h = c // 4, c % 4
        in_maps.append({
            "wte": wte,
            "idx2": idx_b[b],
            "wx": wx_h[h],
            "wy": wy_h[h],
            "enc": enc_h[h],
            "ro": ro_h[h],
            "cs": cs,
            "masks": masks,
            "ident": ident,
        })

    return in_maps


if __name__ == "__main__":
    nc = build_nc()
    print("built + compiled OK")


# revision 39
# speedup vs baseline: 1.0749x; 1.0010x over previous
"""Trainium2 Bass kernel for the BDH-style weight-tied transformer.

Contract: kernel(**inputs) takes FULL unsharded numpy inputs (idx, wte,
encoder, decoder_x, decoder_y, readout) and returns the FULL (B, T, V)
logits, running the model on 8 NeuronCores via run_bass_kernel_spmd.

Sharding: core c -> (b = c // 4, h = c % 4).  Group {0..3} handles batch 0,
{4..7} batch 1.  Within a group: tensor-parallel over heads with AllGather
+ local-sum for (a) the head-summed attention matrix and (b) the second
half of the y @ encoder projection (the first half goes out early as an
AllReduce that hides under the remaining y-phase compute).  LayerNorm is
scale-invariant, so summing heads (instead of averaging) is exact.
Readout is vocab-split 4 ways per group.

The neuron axis of each head is permuted host-side so RoPE pair partners
(2k, 2k+1) live at the same partition of sibling 128-chunks ("even" chunk
2c / "odd" chunk 2c+1).  The rotation then needs no cross-partition data
movement.  The 1/sqrt(d) attention scale is folded into the cos/sin tables
(d**-0.25 on each factor of the Gram matrix).
"""

import sys

for _p in ("/opt/trn_rl_repo", "/opt/pypackages"):
    if _p not in sys.path:
        sys.path.append(_p)

import ml_dtypes
import numpy as np

import concourse.bass as bass
import concourse.mybir as mybir
import concourse.tile as tile
from concourse import bacc
from concourse.bass_utils import run_bass_kernel_spmd

F32 = mybir.dt.float32
BF16 = mybir.dt.bfloat16
I32 = mybir.dt.int32
AX = mybir.AxisListType
ALU = mybir.AluOpType
ACT = mybir.ActivationFunctionType

# Model dims (hardcoded per problem spec)
B, T, D, H, N, V = 2, 256, 256, 4, 32768, 32000
n_head = N // H            # 8192 neurons per head (one core's slice)
P = 128
NCH = n_head // P          # 64 chunks of 128 neurons
NPAIR = NCH // 2           # 32 pair-chunks
L_LAYERS = 6
LN_EPS = 1e-5
ROPE_BASE = 10000.0
VSLICE = V // 4            # 8000 vocab columns per core
VCH = 500                  # vocab chunk (PSUM bank holds 512 f32)
NVCH = VSLICE // VCH       # 16
GX = 8                     # n-chunks per streamed weight group (= rope batch)
NGRP = NCH // GX           # 8 weight groups per phase
S4 = float(n_head) ** -0.25


def _ln_pair(nc, pools, srcs, out):
    """LayerNorm over the free dim (D=256) of two [128, 256] f32 tiles.

    srcs: list of 2 APs (SBUF or PSUM, f32).  out: [128, 2, 256] tile.
    """
    psmall = pools["small"]
    for i, src in enumerate(srcs):
        stats = psmall.tile([P, 6], F32, name=f"ln_st{i}", tag="lnstat")
        nc.vector.bn_stats(stats, src)
        aggr = psmall.tile([P, 2], F32, name=f"ln_ag{i}", tag="lnstat")
        nc.vector.bn_aggr(aggr, stats)
        std = psmall.tile([P, 1], F32, name=f"ln_std{i}", tag="lnstat")
        nc.scalar.activation(std, aggr[:, 1:2], ACT.Sqrt,
                             bias=pools["eps"][:, :1])
        rinv = psmall.tile([P, 1], F32, name=f"ln_rinv{i}", tag="lnstat")
        nc.vector.reciprocal(rinv, std)
        nc.vector.tensor_scalar(out[:, i, :], src, aggr[:, 0:1], rinv,
                                op0=ALU.subtract, op1=ALU.mult)


def _transpose4(nc, pools, src, dst, ident):
    """dst[:, k, 128*i:128*(i+1)] = src[:, i, 128*k:128*(k+1)].T  (bf16).

    src, dst: [128, 2, 256] bf16.  Four PE transposes + ACT copies.
    """
    pwork = pools["ps_work"]
    for i in range(2):
        for k in range(2):
            tp = pwork.tile([P, P], BF16, name=f"tp_{i}_{k}", tag="work")
            nc.tensor.transpose(tp, src[:, i, P * k:P * (k + 1)], ident)
            nc.scalar.copy(dst[:, k, P * i:P * (i + 1)], tp)


def build_nc(num_cores=8):
    nc = bacc.Bacc(
        "TRN2", target_bir_lowering=False, debug=False, num_devices=num_cores
    )

    # ---- DRAM I/O (per-core data supplied via in_maps) ----
    wte_d = nc.dram_tensor("wte", [V, D], F32, kind="ExternalInput").ap()
    idx_d = nc.dram_tensor("idx2", [2, P], I32, kind="ExternalInput").ap()
    wx_d = nc.dram_tensor("wx", [D, n_head], BF16, kind="ExternalInput").ap()
    wy_d = nc.dram_tensor("wy", [D, n_head], BF16, kind="ExternalInput").ap()
    enc_d = nc.dram_tensor("enc", [n_head, D], BF16, kind="ExternalInput").ap()
    ro_d = nc.dram_tensor("ro", [D, VSLICE], BF16, kind="ExternalInput").ap()
    cs_d = nc.dram_tensor("cs", [P, NPAIR, 2 * T], BF16, kind="ExternalInput").ap()
    masks_d = nc.dram_tensor("masks", [P, 2, T], BF16, kind="ExternalInput").ap()
    ident_d = nc.dram_tensor("ident", [P, P], BF16, kind="ExternalInput").ap()
    out_d = nc.dram_tensor("out", [T, VSLICE], F32, kind="ExternalOutput").ap()

    groups = [[0, 1, 2, 3], [4, 5, 6, 7]]

    from contextlib import ExitStack
    with tile.TileContext(nc) as tc:
        with ExitStack() as _stk:
            _e = _stk.enter_context
            pers = _e(tc.tile_pool(name="pers", bufs=1))
            pv = _e(tc.tile_pool(name="pv", bufs=2))
            pbig = _e(tc.tile_pool(name="pbig", bufs=2))
            pwx = _e(tc.tile_pool(name="pwx", bufs=4))
            pwy = _e(tc.tile_pool(name="pwy", bufs=4))
            pro = _e(tc.tile_pool(name="pro", bufs=7))
            pxr = _e(tc.tile_pool(name="pxr", bufs=2))
            py = _e(tc.tile_pool(name="py", bufs=4))
            psmall = _e(tc.tile_pool(name="psmall", bufs=16))
            pexp = _e(tc.tile_pool(name="pexp", bufs=2))
            pg = _e(tc.tile_pool(name="pg", bufs=5))
            ps_work = _e(tc.tile_pool(name="ps_work", bufs=5, space="PSUM"))
            ps_accum = _e(tc.tile_pool(name="ps_accum", bufs=1, space="PSUM"))
            dram = _e(tc.tile_pool(name="dram", bufs=2, space="DRAM"))
            pools = {
                "small": psmall,
                "ps_work": ps_work,
            }

            # ---- persistent SBUF tensors ----
            eps_sb = pers.tile([P, 1], F32, name="eps_sb", tag="eps")
            nc.vector.memset(eps_sb, LN_EPS)
            pools["eps"] = eps_sb
            cs_sb = pers.tile([P, NPAIR, 2 * T], BF16, name="cs_sb", tag="cs")
            masks_sb = pers.tile([P, 2, T], BF16, name="masks_sb", tag="masks")
            ident_sb = pers.tile([P, P], BF16, name="ident_sb", tag="ident")
            enc_sb = pers.tile([P, NCH, T], BF16, name="enc_sb", tag="enc")
            x_sb = pers.tile([P, NCH, T], BF16, name="x_sb", tag="x")

            # ---- embedding gather first: it heads the layer-0 critical path
            # (no warm-up collectives: the ncfw global init (~65us wall) hides
            # under the layer-0 x phase before the first real AllGather)
            vraw = pbig.tile([P, 2, T], F32, name="vraw", tag="vraw", bufs=1)
            for i in range(2):
                idx_sb = psmall.tile([P, 1], I32, name=f"idx_sb{i}", tag="idx")
                nc.sync.dma_start(idx_sb, idx_d[i, :].rearrange("(p o) -> p o", o=1))
                nc.gpsimd.indirect_dma_start(
                    out=vraw[:, i, :],
                    out_offset=None,
                    in_=wte_d[:],
                    in_offset=bass.IndirectOffsetOnAxis(ap=idx_sb[:, :1], axis=0),
                )
            nc.sync.dma_start(masks_sb[:], masks_d[:])
            nc.sync.dma_start(ident_sb[:], ident_d[:])

            def load_cs_group(g):
                # cos/sin for rope batch g (4 pair-chunks), split 4 ways
                for c in range(4):
                    nc.sync.dma_start(
                        cs_sb[:, 4 * g + c:4 * g + c + 1, :],
                        cs_d[:, 4 * g + c:4 * g + c + 1, :],
                    )

            def load_enc_group(g):
                # encoder chunks 8g..8g+7, split 2 ways
                for c in range(2):
                    nc.sync.dma_start(
                        enc_sb[:, 8 * g + 4 * c:8 * g + 4 * (c + 1), :],
                        enc_r[:, 8 * g + 4 * c:8 * g + 4 * (c + 1), :],
                    )

            enc_r = enc_d.rearrange("(c p) d -> p c d", p=P)
            load_cs_group(0)
            load_cs_group(1)

            _kw_id = [0]

            def pe_keepwarm(n=40):
                # Dependency-free matmuls that run during a collective stall so
                # the HAM clock gate keeps the PE at 2.4 GHz (a >3.4us idle
                # re-throttles it to 1.2 GHz for the next ~3.4us of work).
                # Writes ride the yeA accumulator's PSUM bank, which is idle at
                # every keepwarm site (between its flushA read and next-layer
                # yenc writes).
                _kw_id[0] += 1
                jp = ps_accum.tile([P, T], F32, name=f"junk_{_kw_id[0]}",
                                   tag="accC")
                for _ in range(n):
                    nc.tensor.matmul(jp[:, 0:T], lhsT=ident_sb,
                                     rhs=masks_sb[:, 1, :],
                                     start=True, stop=True)

            pe_keepwarm(40)
            v = pv.tile([P, 2, T], F32, name="v_l0", tag="v")
            _ln_pair(nc, pools, [vraw[:, 0, :], vraw[:, 1, :]], v)

            for layer in range(L_LAYERS):
                # ---- v_bf (natural, bf16) and vT (transposed, bf16) ----
                v_bf = pbig.tile([P, 2, T], BF16, name=f"vbf_{layer}", tag="vbf")
                for i in range(2):
                    nc.scalar.copy(v_bf[:, i, :], v[:, i, :])
                vT = pbig.tile([P, 2, T], BF16, name=f"vT_{layer}", tag="vT")
                _transpose4(nc, pools, v_bf, vT, ident_sb)

                # ---- x phase: x = relu(v @ Wx), rope, scores (Gram) ----
                # sc0/sc1 accumulation groups interleave, so they must live in
                # different PSUM banks (start=True owns a whole 2KB zero region)
                sc0 = ps_accum.tile([P, P], F32, name=f"sc0_{layer}", tag="accA")
                sc1 = ps_accum.tile([P, T], F32, name=f"sc1_{layer}", tag="accB")
                scores = [sc0, sc1]

                def emit_scores(grp, xr_e, xr_o):
                    ch0 = GX * grp
                    for q in range(4):  # pair within rope batch
                        for xr in (xr_e, xr_o):
                            chv = ch0 + 2 * q + (0 if xr is xr_e else 1)
                            nc.tensor.matmul(
                                scores[0],
                                lhsT=xr[:, q, 0:P],
                                rhs=xr[:, q, 0:P],
                                start=(chv == 0),
                                stop=(chv == NCH - 1),
                            )
                            nc.tensor.matmul(
                                scores[1],
                                lhsT=xr[:, q, P:2 * P],
                                rhs=xr[:, q, :],
                                start=(chv == 0),
                                stop=(chv == NCH - 1),
                            )

                pending = []  # (grp, xr_e, xr_o) awaiting scores emission
                for grp in range(NGRP):  # 8 groups of 8 chunks (4 rope pairs)
                    ch0 = GX * grp
                    wxg = pwx.tile([P, 2, GX * P], BF16,
                                   name=f"wxg_{layer}_{ch0}", tag="wx")
                    for dk in range(2):
                        for ch in range(2):
                            nc.sync.dma_start(
                                wxg[:, dk, 512 * ch:512 * (ch + 1)],
                                wx_d[P * dk:P * (dk + 1),
                                     P * ch0 + 512 * ch:P * ch0 + 512 * (ch + 1)],
                            )
                    if layer == 0:
                        if grp + 2 < NGRP:
                            load_cs_group(grp + 2)
                    for pc in range(4 * grp, 4 * grp + 4):
                        x_pre = ps_work.tile([P, 2 * T], F32,
                                             name=f"xpre_{layer}_{pc}", tag="work")
                        for m in range(2):  # even / odd member chunk
                            ch = 2 * pc + m
                            co = P * (ch % GX)
                            for dk in range(2):
                                nc.tensor.matmul(
                                    x_pre[:, T * m:T * (m + 1)],
                                    lhsT=wxg[:, dk, co:co + P],
                                    rhs=vT[:, dk, :],
                                    start=(dk == 0),
                                    stop=(dk == 1),
                                )
                        nc.scalar.activation(
                            x_sb[:, 2 * pc:2 * pc + 2, :], x_pre, ACT.Relu)
                    # rope over the 4 pair-chunks of this group, batched FD=1024
                    xe = x_sb[:, ch0:ch0 + GX:2, :]
                    xo = x_sb[:, ch0 + 1:ch0 + GX:2, :]
                    cvw = cs_sb[:, 4 * grp:4 * grp + 4, 0:T]
                    svw = cs_sb[:, 4 * grp:4 * grp + 4, T:2 * T]
                    m_ec = pxr.tile([P, 4, T], BF16, name=f"mec_{layer}_{grp}", tag="m_ec", bufs=1)
                    m_os = pxr.tile([P, 4, T], BF16, name=f"mos_{layer}_{grp}", tag="m_os", bufs=1)
                    m_oc = pxr.tile([P, 4, T], BF16, name=f"moc_{layer}_{grp}", tag="m_oc", bufs=1)
                    m_es = pxr.tile([P, 4, T], BF16, name=f"mes_{layer}_{grp}", tag="m_es", bufs=1)
                    xr_e = pxr.tile([P, 4, T], BF16, name=f"xre_{layer}_{grp}", tag="xr_e", bufs=3)
                    xr_o = pxr.tile([P, 4, T], BF16, name=f"xro_{layer}_{grp}", tag="xr_o", bufs=3)
                    nc.vector.tensor_mul(m_ec, xe, cvw)
                    nc.vector.tensor_mul(m_os, xo, svw)
                    nc.vector.tensor_sub(xr_e, m_ec, m_os)
                    nc.vector.tensor_mul(m_oc, xo, cvw)
                    nc.vector.tensor_mul(m_es, xe, svw)
                    nc.vector.tensor_add(xr_o, m_oc, m_es)
                    # 2-group pipeline depth: the relu->rope chain latency
                    # (~5.7us) exceeds the PE group period (~4us), so scores
                    # for group g are emitted while group g+2 computes
                    pending.append((grp, xr_e, xr_o))
                    if len(pending) > 2:
                        emit_scores(*pending.pop(0))
                for p in pending:
                    emit_scores(*p)
                pe_keepwarm(6)  # PE filler while softmax runs

                # ---- softmax (causal, per-head normalized) ----
                # attn packed [128, 384]: cols 0:128 = t-tile0 (s<128),
                # cols 128:384 = t-tile1 (s<256)
                attn = pexp.tile([P, 3 * P], BF16, name=f"attn_{layer}", tag="attn")
                for i, (w, lo) in enumerate(((P, 0), (T, P))):
                    # scores are pre-scaled by 1/sqrt(d) via the rope tables and
                    # bounded (~|s|<15 for this weight scale), so exp needs no
                    # max-subtraction; softmax is shift-invariant.  The causal
                    # mask is additive (-30000 on masked entries, exp -> 0) and
                    # the row-sum rides the exp via accum_out.
                    nc.vector.tensor_add(scores[i], scores[i], masks_sb[:, i, 0:w])
                    ex = pexp.tile([P, w], BF16, name=f"ex_{layer}_{i}", tag="ex")
                    rs = psmall.tile([P, 1], F32, name=f"rs_{i}", tag="lnstat")
                    nc.scalar.activation(ex, scores[i], ACT.Exp, accum_out=rs)
                    rcp = psmall.tile([P, 1], F32, name=f"rcp_{i}", tag="lnstat")
                    nc.vector.reciprocal(rcp, rs)
                    nc.vector.tensor_scalar_mul(attn[:, lo:lo + w], ex, rcp)

                # ---- transpose local attn (pre-collective), AllGather over
                # the 4-core group, then sum heads locally ----
                attnT = pexp.tile([P, 3 * P], BF16, name=f"attnT_{layer}", tag="attnT")
                for bi, (alo, tlo) in enumerate(((0, 0), (P, P), (2 * P, 2 * P))):
                    tp = ps_work.tile([P, P], BF16, name=f"tpa_{bi}", tag="work")
                    nc.tensor.transpose(tp, attn[:, alo:alo + P], ident_sb)
                    nc.scalar.copy(attnT[:, tlo:tlo + P], tp)
                attn_bnc = dram.tile([P, 3 * P], BF16,
                                     name=f"attn_bnc_{layer}", tag="attn_in")
                nc.gpsimd.dma_start(attn_bnc[:, 0:P], attnT[:, 0:P])
                nc.scalar.dma_start(attn_bnc[:, P:3 * P], attnT[:, P:3 * P])
                attn_gth = dram.tile([4, P, 3 * P], BF16, name=f"attn_gth_{layer}",
                                     tag="attn_out")
                nc.gpsimd.collective_compute(
                    "AllGather", ALU.bypass, replica_groups=groups,
                    ins=[attn_bnc.opt()], outs=[attn_gth.opt()],
                )
                pe_keepwarm(35)
                ga = []
                for r in range(4):
                    g_r = pg.tile([P, 3 * P], BF16, name=f"ga_{layer}_{r}", tag="ag")
                    nc.sync.dma_start(g_r, attn_gth[r])
                    ga.append(g_r)
                s01 = pg.tile([P, 3 * P], BF16, name=f"s01_{layer}", tag="ag")
                nc.vector.tensor_add(s01, ga[0], ga[1])
                s23 = pg.tile([P, 3 * P], BF16, name=f"s23_{layer}", tag="ag")
                nc.vector.tensor_add(s23, ga[2], ga[3])
                asumT = pexp.tile([P, 3 * P], BF16, name=f"asumT_{layer}", tag="asum")
                nc.vector.tensor_add(asumT, s01, s23)

                # ---- a = asumT.T @ v; LN(a) ----
                a_ps = []
                ap_0 = ps_work.tile([P, T], F32, name=f"aps_{layer}_0", tag="work")
                nc.tensor.matmul(ap_0, lhsT=asumT[:, 0:P], rhs=v_bf[:, 0, :],
                                 start=True, stop=True)
                a_ps.append(ap_0)
                ap_1 = ps_work.tile([P, T], F32, name=f"aps_{layer}_1", tag="work")
                for j in range(2):
                    nc.tensor.matmul(
                        ap_1,
                        lhsT=asumT[:, P * (1 + j):P * (2 + j)],
                        rhs=v_bf[:, j, :],
                        start=(j == 0),
                        stop=(j == 1),
                    )
                a_ps.append(ap_1)
                lnA = pbig.tile([P, 2, T], BF16, name=f"lnA_{layer}", tag="lnA")
                _ln_pair(nc, pools, a_ps, lnA)
                lnAT = pbig.tile([P, 2, T], BF16, name=f"lnAT_{layer}", tag="lnAT")
                _transpose4(nc, pools, lnA, lnAT, ident_sb)

                # ---- y phase: y = relu(lnA @ Wy) * x (fused DVE op);
                # yenc = y @ enc accumulated over all chunks, then one
                # AllReduce over the 4-core group ----
                ye0 = ps_accum.tile([P, T], F32, name=f"ye0_{layer}", tag="accA")
                ye1 = ps_accum.tile([P, T], F32, name=f"ye1_{layer}", tag="accB")
                yacc = (ye0, ye1)

                def emit_yenc(pc, yt):
                    ch0y = 2 * pc
                    for m in range(2):
                        ch = ch0y + m
                        for i in range(2):
                            nc.tensor.matmul(
                                yacc[i],
                                lhsT=yt[:, T * m + P * i:T * m + P * (i + 1)],
                                rhs=enc_sb[:, ch, :],
                                start=(ch == 0),
                                stop=(ch == NCH - 1),
                            )

                pend_y = []
                for pc in range(NCH // 2):  # two n-chunks at a time
                    ch0y = 2 * pc
                    if ch0y % GX == 0:
                        gy = ch0y // GX
                        wyg = pwy.tile([P, 2, GX * P], BF16,
                                       name=f"wyg_{layer}_{ch0y}", tag="wy")
                        for dk in range(2):
                            for ch in range(2):
                                nc.sync.dma_start(
                                    wyg[:, dk, 512 * ch:512 * (ch + 1)],
                                    wy_d[P * dk:P * (dk + 1),
                                         P * ch0y + 512 * ch:P * ch0y + 512 * (ch + 1)],
                                )
                        if layer == 0 and gy == 0:
                            load_enc_group(0)
                        if layer == 0 and gy + 1 < NGRP:
                            load_enc_group(gy + 1)
                    y_pre = ps_work.tile([P, 2 * T], F32, name=f"ypre_{layer}_{pc}",
                                         tag="work")
                    for m in range(2):
                        co = P * ((ch0y + m) % GX)
                        for dk in range(2):
                            nc.tensor.matmul(
                                y_pre[:, T * m:T * (m + 1)],
                                lhsT=wyg[:, dk, co:co + P],
                                rhs=lnAT[:, dk, :],
                                start=(dk == 0),
                                stop=(dk == 1),
                            )
                    # fused y = relu(y_pre) * x in one DVE op; yenc emission
                    # lags 2 chunks so the PE->DVE->PE chain stays pipelined
                    yt = py.tile([P, 2 * T], BF16, name=f"yt_{layer}_{pc}", tag="y")
                    nc.vector.scalar_tensor_tensor(
                        yt, y_pre, 0.0, x_sb[:, ch0y:ch0y + 2, :].rearrange("p c t -> p (c t)"),
                        op0=ALU.max, op1=ALU.mult)
                    pend_y.append((pc, yt))
                    if len(pend_y) > 2:
                        emit_yenc(*pend_y.pop(0))
                for p in pend_y:
                    emit_yenc(*p)

                # ---- AllReduce yenc partials; bounce split across the idle
                # gpsimd + scalar DMA queues (sync queues hold the wy tail) ----
                ye_sb = pexp.tile([P, 2 * T], BF16, name=f"ye_sb_{layer}", tag="yeA")
                for i in range(2):
                    nc.scalar.copy(ye_sb[:, T * i:T * (i + 1)], yacc[i])
                ye_bnc = dram.tile([P, 2 * T], BF16, name=f"ye_bnc_{layer}",
                                   tag="yeA_in")
                nc.gpsimd.dma_start(ye_bnc[:, 0:T], ye_sb[:, 0:T])
                nc.scalar.dma_start(ye_bnc[:, T:2 * T], ye_sb[:, T:2 * T])
                ye_gth = dram.tile([P, 2 * T], BF16, name=f"ye_gth_{layer}",
                                   tag="yeA_out")
                nc.gpsimd.collective_compute(
                    "AllReduce", ALU.add, replica_groups=groups,
                    ins=[ye_bnc.opt()], outs=[ye_gth.opt()],
                )
                pe_keepwarm(45)
                ysum = pbig.tile([P, 2, T], BF16, name=f"ysum_{layer}", tag="ysum")
                for i in range(2):  # split per t-tile so lnY(t0) starts early
                    nc.sync.dma_start(ysum[:, i, :], ye_gth[:, T * i:T * (i + 1)])
                lnY = pbig.tile([P, 2, T], F32, name=f"lnY_{layer}", tag="lnY", bufs=1)
                _ln_pair(nc, pools, [ysum[:, 0, :], ysum[:, 1, :]], lnY)
                vres = pbig.tile([P, 2, T], F32, name=f"vres_{layer}", tag="vres", bufs=1)
                for i in range(2):
                    nc.vector.tensor_add(vres[:, i, :], v[:, i, :], lnY[:, i, :])
                v = pv.tile([P, 2, T], F32, name=f"v_l{layer + 1}", tag="v")
                _ln_pair(nc, pools, [vres[:, 0, :], vres[:, 1, :]], v)

            # ---- readout: out = v @ ro  (vocab slice) ----
            v_bf = pbig.tile([P, 2, T], BF16, name="vbf_ro", tag="vbf")
            for i in range(2):
                nc.scalar.copy(v_bf[:, i, :], v[:, i, :])
            vT = pbig.tile([P, 2, T], BF16, name="vT_ro", tag="vT")
            _transpose4(nc, pools, v_bf, vT, ident_sb)
            for c in range(NVCH):
                rog = pro.tile([P, 2, VCH], BF16, name=f"rog_{c}", tag="ro")
                for dk in range(2):
                    nc.sync.dma_start(
                        rog[:, dk, :],
                        ro_d[P * dk:P * (dk + 1), VCH * c:VCH * (c + 1)],
                    )
                for i in range(2):
                    lg = ps_work.tile([P, VCH], F32, name=f"lg_{c}_{i}", tag="work")
                    for dk in range(2):
                        nc.tensor.matmul(
                            lg,
                            lhsT=vT[:, dk, P * i:P * (i + 1)],
                            rhs=rog[:, dk, :],
                            start=(dk == 0),
                            stop=(dk == 1),
                        )
                    lg_sb = py.tile([P, VCH], F32, name=f"lg_sb_{c}_{i}",
                                    tag="lgsb", bufs=4)
                    nc.vector.tensor_copy(lg_sb, lg)
                    nc.sync.dma_start(
                        out_d[P * i:P * (i + 1), VCH * c:VCH * (c + 1)], lg_sb
                    )

    nc.compile()
    return nc


# ------------------------- host-side preparation -------------------------

def _pair_perm():
    """perm[new] = old index within a head, de-interleaving rope pairs."""
    perm = np.zeros(n_head, dtype=np.int64)
    for c in range(NPAIR):
        k = np.arange(P) + c * P          # pair indices in this pair-chunk
        perm[(2 * c) * P + np.arange(P)] = 2 * k
        perm[(2 * c + 1) * P + np.arange(P)] = 2 * k + 1
    return perm


def _rope_tables():
    """cs[p, c, 0:T] = cos, cs[p, c, T:2T] = sin, scaled by d**-0.25."""
    inv_freq = 1.0 / (
        ROPE_BASE ** (np.arange(0, n_head, 2, dtype=np.float32) / n_head)
    )  # (4096,) f32, matching reference arithmetic
    t = np.arange(T, dtype=np.float32)
    freqs = t[:, None] * inv_freq[None, :]         # (T, 4096) f32
    cos = np.cos(freqs) * S4                       # (T, 4096)
    sin = np.sin(freqs) * S4
    cs = np.zeros((P, NPAIR, 2 * T), dtype=np.float32)
    for c in range(NPAIR):
        k = c * P + np.arange(P)                   # (128,) pair indices
        cs[:, c, 0:T] = cos[:, k].T
        cs[:, c, T:2 * T] = sin[:, k].T
    return cs.astype(ml_dtypes.bfloat16)


def _masks():
    # Additive causal mask [P, 2, T]: 0 on allowed entries, -30000 on masked
    # (exp underflows to 0).  tile0 in [:, 0, 0:128] (s<=t); tile1 [:, 1, :].
    m = np.full((P, 2, T), -30000.0, dtype=np.float32)
    t = np.arange(P)[:, None]
    m[:, 0, 0:P] = np.where(np.arange(P)[None, :] <= t, 0.0, -30000.0)
    m[:, 1, :] = np.where(np.arange(T)[None, :] <= t + P, 0.0, -30000.0)
    return m.astype(ml_dtypes.bfloat16)


_CACHE = {}


def kernel(idx, wte, encoder, decoder_x, decoder_y, readout):
    if "nc" not in _CACHE:
        _CACHE["nc"] = build_nc()
    nc = _CACHE["nc"]
    in_maps = prepare_in_maps(idx, wte, encoder, decoder_x, decoder_y, readout)
    res = run_bass_kernel_spmd(nc, in_maps, core_ids=list(range(8)))
    return assemble_output([res.results[c]["out"] for c in range(8)])


def assemble_output(outs):
    out = np.empty((B, T, V), dtype=np.float32)
    for c in range(8):
        b, h = c // 4, c % 4
        out[b, :, h * VSLICE:(h + 1) * VSLICE] = outs[c]
    return out


def prepare_in_maps(idx, wte, encoder, decoder_x, decoder_y, readout):
    idx = np.asarray(idx)
    wte = np.ascontiguousarray(np.asarray(wte, dtype=np.float32))
    encoder = np.asarray(encoder, dtype=np.float32)
    decoder_x = np.asarray(decoder_x, dtype=np.float32)
    decoder_y = np.asarray(decoder_y, dtype=np.float32)
    readout = np.asarray(readout, dtype=np.float32)

    perm = _pair_perm()
    cs = _rope_tables()
    masks = _masks()
    ident = np.eye(P, dtype=np.float32).astype(ml_dtypes.bfloat16)
    bf = ml_dtypes.bfloat16

    wx_h = [np.ascontiguousarray(decoder_x[h][:, perm].astype(bf)) for h in range(H)]
    wy_h = [np.ascontiguousarray(decoder_y[h][:, perm].astype(bf)) for h in range(H)]
    enc_h = [
        np.ascontiguousarray(encoder[h * n_head + perm, :].astype(bf))
        for h in range(H)
    ]
    ro_h = [
        np.ascontiguousarray(readout[:, h * VSLICE:(h + 1) * VSLICE].astype(bf))
        for h in range(H)
    ]
    idx_b = [np.ascontiguousarray(idx[b].reshape(2, P).astype(np.int32))
             for b in range(B)]

    in_maps = []
    for c in range(8):
        b, h = c // 4, c % 4
        in_maps.append({
            "wte": wte,
            "idx2": idx_b[b],
            "wx": wx_h[h],
            "wy": wy_h[h],
            "enc": enc_h[h],
            "ro": ro_h[h],
            "cs": cs,
            "masks": masks,
            "ident": ident,
        })

    return in_maps


if __name__ == "__main__":
    nc = build_nc()
    print("built + compiled OK")
